# revision 10
# baseline (speedup 1.0000x reference)
"""Capsule routing pooling kernel for Trainium2 (8 NeuronCores, data parallel).

Math: the reference's softmax is over a singleton axis, so the routing
coefficients are identically 1.0 and the routing iterations never affect the
output.  The computation reduces to, per (b, c, 2x2 spatial tile):
    s   = sum of the four D=16 vectors in the tile
    sq  = sum_d s_d^2
    out = s * sqrt(sq) / (1 + sq)

Sharding: batch dim (16) split across 8 cores -> 2 batches/core.  Per core the
(2*64)=128 (b,c) pairs map onto the 128 SBUF partitions; each partition owns a
full 64x64x16 image.

v4 pipeline (bf16 datapath; rel-err budget is 2e-2, bf16 noise is ~4e-3):
  - all loads are SWDGE (gpsimd) DMAs casting f32 -> bf16 in the DMA
    datapath: HBM read bytes unchanged (the floor, ~96us at measured
    333 GB/s), SBUF writes halved, and DVE tensor_tensor ops get the bf16
    2x_1P perf mode.  Loads are 2MB units (16KB/partition reads) to
    minimize Q7 descriptor-emission time, which gates the pipeline start.
  - pooled sums live in ONE persistent SBUF buffer (32KB/partition) instead
    of a recycled pool: fronts never block on store completion, so the load
    stream never stalls (v3's drain was fronts waiting on slow stores)
  - super-groups of 4 row-pairs so the PSUM square tile (8KB/partition)
    double-buffers: ACT squares SG i+1 while DVE reduces SG i
  - tails: ACT square -> DVE reduce over D -> scale = sqrt(sq) * 1/(1+sq)
    (ACT sqrt + DVE fast-reciprocal) -> DVE broadcast multiply in place
  - stores are decoupled from tails and batched to ~1MB (8KB/partition) on
    the otherwise-idle sync HWDGE ring; big store packets take few SDMA
    round-robin slots from the load queue
  - output dram tensor is bf16 (half the store traffic); host upcasts
"""

import numpy as np

import concourse.bass as bass
import concourse.bacc as bacc
import concourse.tile as tile
from concourse import mybir
from concourse.bass_utils import run_bass_kernel_spmd

_B, _C, _H, _W, _D = 16, 64, 64, 64, 16
_NCORES = 8
_F32 = mybir.dt.float32
_BF16 = mybir.dt.bfloat16


def _kernel_body(tc, out_ap, in_ap, H, W, D):
    nc = tc.nc
    P = 128
    nH, nW = H // 2, W // 2

    inv4 = in_ap.rearrange("p (q four) w d -> p q (four w d)", four=4)
    inv2 = in_ap.rearrange("p (rp two) w d -> p rp (two w d)", two=2)
    outv = out_ap.rearrange("p y x d -> p y (x d)")

    # super-group schedule in row-pair units: 4s in the bulk (PSUM
    # double-buffering wants nsg <= 128), tapering at the end so the owed
    # tail work stays under the remaining load-stream time and the
    # post-last-load drain chain is short
    if nH >= 16:
        sched = [4] * ((nH - 8) // 4) + [2, 2, 2, 1, 1]
    else:
        sched = [nH]
    assert sum(sched) == nH
    nsg_max = max(sched) * nW

    import contextlib

    with contextlib.ExitStack() as ctx:
        slabs = ctx.enter_context(tc.tile_pool(name="slabs", bufs=8))
        rpool = ctx.enter_context(tc.tile_pool(name="rpool", bufs=3))
        sall_pool = ctx.enter_context(tc.tile_pool(name="sall", bufs=1))
        psum = ctx.enter_context(tc.tile_pool(name="psum", bufs=2, space="PSUM"))
        small = ctx.enter_context(tc.tile_pool(name="small", bufs=3))
        smallb = ctx.enter_context(tc.tile_pool(name="smallb", bufs=3))

        # persistent pooled-sum buffer for the whole image (bf16, 32KB/part)
        sall = sall_pool.tile([P, nH, nW, D], _BF16, tag="sall")

        def emit_front(sg, g0):
            """loads + row-pair adds + column-pair add for one super-group of
            `sg` row-pairs starting at output row g0; result lands in
            sall[:, g0:g0+sg]."""
            r = rpool.tile([P, 4, nW, 2, D], _BF16, tag="r")
            for li in range(0, sg, 2):
                if sg - li >= 2:
                    assert (g0 + li) % 2 == 0, "coarse slab needs even alignment"
                    t = (g0 + li) // 2
                    slab = slabs.tile([P, 2, 2, nW, 2, D], _BF16, tag="slab")
                    nc.gpsimd.dma_start(
                        out=slab[:],
                        in_=inv4[:, t, :].rearrange(
                            "p (a two b) -> p a two b", a=2, two=2
                        ),
                    )
                    # row-pair sums for 2 row-pairs (DVE bf16 2x, FD=2048)
                    nc.vector.tensor_add(
                        r[:, li : li + 2, :, :, :],
                        slab[:, :, 0, :, :, :],
                        slab[:, :, 1, :, :, :],
                    )
                else:
                    rp = g0 + li
                    slab = slabs.tile([P, 1, 2, nW, 2, D], _BF16, tag="slab")
                    nc.gpsimd.dma_start(
                        out=slab[:],
                        in_=inv2[:, rp, :].rearrange("p (two b) -> p two b", two=2),
                    )
                    nc.vector.tensor_add(
                        r[:, li : li + 1, :, :, :],
                        slab[:, :, 0, :, :, :],
                        slab[:, :, 1, :, :, :],
                    )
            # column-pair add (DVE bf16 2x)
            nc.vector.tensor_add(
                sall[:, g0 : g0 + sg, :, :],
                r[:, 0:sg, :, 0, :],
                r[:, 0:sg, :, 1, :],
            )

        def chain_views(nsg):
            ch = small.tile([P, nsg_max, 3], _F32, tag="ch")
            scb = smallb.tile([P, nsg_max, 1], _BF16, tag="scb")
            sq = ch[:, 0:nsg, 0:1]
            a = ch[:, 0:nsg, 1:2]
            rec = ch[:, 0:nsg, 2:3]
            sc = scb[:, 0:nsg, 0:1]
            return sq, a, rec, sc

        def emit_tail(sg, g0):
            """square + reduce + squash scale + in-place broadcast multiply
            for rows [g0, g0+sg) of sall."""
            nsg = sg * nW
            sv = sall[:, g0 : g0 + sg, :, :].rearrange("p s x d -> p (s x) d")
            s2p = psum.tile([P, nsg_max, D], _F32, tag="s2p")
            nc.scalar.activation(
                s2p[:, 0:nsg, :], sv, mybir.ActivationFunctionType.Square
            )
            sq, a, rec, sc = chain_views(nsg)
            nc.vector.tensor_reduce(
                sq, s2p[:, 0:nsg, :], axis=mybir.AxisListType.X, op=mybir.AluOpType.add
            )
            # scale = sqrt(sq) / (1 + sq)   (1e-8 dropped: sq >= O(1) for
            # this distribution; relative effect <= 1e-6)
            nc.scalar.activation(a, sq, mybir.ActivationFunctionType.Sqrt)
            nc.scalar.add(rec, sq, 1.0)
            nc.vector.reciprocal_approx_fast(rec, rec)
            nc.vector.tensor_mul(sc, a, rec)
            # out = s * scale (broadcast over D), in place on sall
            nc.vector.tensor_mul(sv, sv, sc.to_broadcast((P, nsg, D)))

        def emit_store(y0, y1):
            nc.sync.dma_start(
                out=outv[:, y0:y1, :],
                in_=sall[:, y0:y1, :, :].rearrange("p s x d -> p (s x d)"),
            )

        def emit_tail2(t1, t2):
            """the last two tails, op-interleaved so ACT and DVE pipeline
            instead of ping-ponging through two serial chains."""
            (sg1, g01), (sg2, g02) = t1, t2
            n1, n2 = sg1 * nW, sg2 * nW
            sv1 = sall[:, g01 : g01 + sg1, :, :].rearrange("p s x d -> p (s x) d")
            sv2 = sall[:, g02 : g02 + sg2, :, :].rearrange("p s x d -> p (s x) d")
            sq1, a1, rec1, sc1 = chain_views(n1)
            sq2, a2, rec2, sc2 = chain_views(n2)
            p1 = psum.tile([P, nsg_max, D], _F32, tag="s2p")
            nc.scalar.activation(
                p1[:, 0:n1, :], sv1, mybir.ActivationFunctionType.Square
            )
            nc.vector.tensor_reduce(
                sq1, p1[:, 0:n1, :], axis=mybir.AxisListType.X, op=mybir.AluOpType.add
            )
            p2 = psum.tile([P, nsg_max, D], _F32, tag="s2p")
            nc.scalar.activation(
                p2[:, 0:n2, :], sv2, mybir.ActivationFunctionType.Square
            )
            nc.scalar.activation(a1, sq1, mybir.ActivationFunctionType.Sqrt)
            nc.scalar.add(rec1, sq1, 1.0)
            nc.vector.reciprocal_approx_fast(rec1, rec1)
            nc.vector.tensor_mul(sc1, a1, rec1)
            nc.vector.tensor_reduce(
                sq2, p2[:, 0:n2, :], axis=mybir.AxisListType.X, op=mybir.AluOpType.add
            )
            nc.vector.tensor_mul(sv1, sv1, sc1.to_broadcast((P, n1, D)))
            nc.scalar.activation(a2, sq2, mybir.ActivationFunctionType.Sqrt)
            nc.scalar.add(rec2, sq2, 1.0)
            nc.vector.reciprocal_approx_fast(rec2, rec2)
            nc.vector.tensor_mul(sc2, a2, rec2)
            nc.vector.tensor_mul(sv2, sv2, sc2.to_broadcast((P, n2, D)))

        g0 = 0
        last = len(sched) - 1
        pending = []  # (sg, g0) awaiting tail
        done_tails = []  # (sg, g0) tails emitted, store pending
        stored_to = 0

        def flush_store(min_rows=8):
            nonlocal stored_to
            done_rows = sum(sg for sg, _ in done_tails)
            if done_tails and done_rows >= min_rows:
                y1 = stored_to + done_rows
                emit_store(stored_to, y1)
                stored_to = y1
                done_tails.clear()

        for si, sg in enumerate(sched):
            emit_front(sg, g0)
            if pending and si < last:
                tsg, tg0 = pending.pop(0)
                emit_tail(tsg, tg0)
                done_tails.append((tsg, tg0))
                # smaller store batches near the end: the final store sits on
                # the critical path after the last bigmul
                flush_store(min_rows=8 if si < len(sched) - 4 else 3)
            pending.append((sg, g0))
            g0 += sg
        if len(pending) == 2:
            emit_tail2(pending[0], pending[1])
            done_tails.extend(pending)
            pending.clear()
        else:
            for t in pending:
                emit_tail(*t)
                done_tails.append(t)
            pending.clear()
        flush_store(min_rows=1)
        assert stored_to == nH


def build_nc(H=_H, W=_W, D=_D):
    """Build and compile the per-core Bass program."""
    nc = bacc.Bacc("TRN2", target_bir_lowering=False, debug=False)
    inp = nc.dram_tensor("inp", [128, H, W, D], _F32, kind="ExternalInput").ap()
    out = nc.dram_tensor(
        "out", [128, H // 2, W // 2, D], _BF16, kind="ExternalOutput"
    ).ap()
    with tile.TileContext(nc) as tc:
        _kernel_body(tc, out, inp, H, W, D)
    nc.compile()
    return nc


_NC_CACHE = {}


def _get_nc():
    if "nc" not in _NC_CACHE:
        _NC_CACHE["nc"] = build_nc()
    return _NC_CACHE["nc"]


def kernel(inp, kernel_size=2, routing_iteration=3, _trace=False, _tmpdir=None):
    inp = np.asarray(inp, dtype=np.float32)
    assert int(kernel_size) == 2, "kernel compiled for kernel_size=2"
    assert inp.shape == (_B, _C, _H, _W, _D), inp.shape
    # routing_iteration is mathematically irrelevant (softmax over singleton
    # axis -> coefficients identically 1); any value >= 1 gives this output.

    nc = _get_nc()
    bpc = _B // _NCORES  # batches per core
    in_maps = [
        {"inp": np.ascontiguousarray(inp[i * bpc : (i + 1) * bpc]).reshape(128, _H, _W, _D)}
        for i in range(_NCORES)
    ]
    res = run_bass_kernel_spmd(
        nc, in_maps, core_ids=list(range(_NCORES)), trace=_trace, tmpdir=_tmpdir
    )
    out = np.empty((_B, _C, _H // 2, _W // 2, _D), dtype=np.float32)
    for i in range(_NCORES):
        out[i * bpc : (i + 1) * bpc] = (
            np.asarray(res.results[i]["out"])
            .astype(np.float32)
            .reshape(bpc, _C, _H // 2, _W // 2, _D)
        )
    if _trace:
        return out, res
    return out
